# revision 7
# baseline (speedup 1.0000x reference)
"""Trainium2 Bass kernel for nn_Attention_56916906606885 (topk channel masking).

Reference computation (per sample b of 32):
  avg[c] = mean(x[b,c,:,:]); mx[c] = max(x[b,c,:,:])          # [512]
  z = conv1d(avg,w,pad=1) + conv1d(mx,w,pad=1)                 # [512] logits
  scores = sigmoid(z)
  top K=256 channels by score, re-sorted by ascending channel index
  out[b,j] = scores[sidx[j]] * x[b, sidx[j]]                   # [256,56,56]

Design (8 NeuronCores, data-parallel over batch, 4 samples/core).
DMA-fabric-bound kernel (16 SDMA engines ~26GB/s each => ~420GB/s/core):
the goal is to keep load DMA (25.7MB f32) and scatter DMA overlapped and
everything else off the critical path.

  - x streamed in [128,3136] channel tiles (recycling pool, 12 bufs).
  - per-channel sum on ScalarE (activation Copy + accum_out), per-channel
    max on VectorE (tensor_reduce) - f32 exact, so the topk selection
    matches the reference bit-for-bit.
  - comb = sum/HW + max written into a zero-guarded padded slab so the
    conv1d boundary terms are plain column-shifted matmul operands - no
    shifted copies needed.
  - z = conv1d(comb) on the PE as tridiagonal matmul + 2 boundary matmuls.
  - selection without sort: rank[i] = #{j: score[j] > score[i]} via DVE
    tensor_scalar against a PE-broadcast score row; mask = rank < K;
    output row = inclusive-prefix-sum of mask (PE matmuls). Unselected
    rows get offsets > bounds so the scatter's OOB-skip drops them.
  - scale pass multiplies by sigmoid score AND downcasts to bf16
    (output tolerance is 2e-2; bf16 rounding is ~1e-3), halving scatter
    bytes. Scales are split across ScalarE/GpSimd to keep VectorE free
    for the 16 max-reduces (its critical load).
  - per-tile indirect DMA scatters selected bf16 rows straight to DRAM;
    host upcasts to f32.
"""

import sys

for _p in ("/opt/trn_rl_repo",):
    if _p not in sys.path:
        sys.path.insert(0, _p)

import numpy as np

import concourse.bass as bass
import concourse.bacc as bacc
import concourse.tile as tile
from concourse import mybir
from concourse.bass_utils import run_bass_kernel_spmd

F32 = mybir.dt.float32
BF16 = mybir.dt.bfloat16
I32 = mybir.dt.int32
AF = mybir.ActivationFunctionType
OP = mybir.AluOpType

B, C, H, W = 32, 512, 56, 56
HW = H * W  # 3136
K = 256
NCORES = 8
SPB = B // NCORES  # 4 samples per core
P = 128
NT = C // P  # 4 channel tiles per sample
FLAT_IN = SPB * C  # 2048 rows per core
BIG = 65536.0  # OOB marker for unselected channels (> any valid row index)
CP = NT + 2  # padded comb columns per sample (zero guard on both sides)

# engine for each of the 16 scale passes, indexed s*NT+t.
# "a"=ScalarE activation (3.0us measured), "g"=GpSimd tensor_tensor with
# stride-0 broadcast score (5.4us measured). VectorE bf16 writes measured
# at 46us (!) so DVE never writes bf16; it keeps the 16 max-reduces.
SCALE_ENG = [
    "a", "g", "g", "a",
    "g", "a", "a", "g",
    "a", "g", "g", "a",
    "g", "a", "a", "g",
]

_CACHE = {}


def build_nc(finalize=True):
    nc = bacc.Bacc()
    x = nc.declare_dram_parameter("x", [FLAT_IN, HW], F32, isOutput=False)
    wt = nc.declare_dram_parameter("w", [1, 3], F32, isOutput=False)
    outs = [
        nc.declare_dram_parameter(f"out{s}", [K, HW], BF16, isOutput=True)
        for s in range(SPB)
    ]

    with tile.TileContext(nc) as tc:
        with (
            tc.tile_pool(name="xp", bufs=12) as xp,
            tc.tile_pool(name="bp", bufs=6) as bp,
            tc.tile_pool(name="small", bufs=1) as sp,
            tc.tile_pool(name="trash", bufs=1) as tp,
            tc.tile_pool(name="psum", bufs=2, space="PSUM") as pp,
            tc.tile_pool(name="psum2", bufs=2, space="PSUM") as pp2,
        ):
            # enqueue every tile load up front: the sync queue is otherwise
            # idle and the DMA engines should never starve. Loads of tiles
            # >= bufs wait (on the sync queue) for the scale pass to free
            # a buffer, which happens long before the data is needed.
            xt_of = {s: [] for s in range(SPB)}
            for s in range(SPB):
                for t in range(NT):
                    xti = xp.tile([P, HW], F32, tag="xt")
                    nc.sync.dma_start(
                        xti[:, :], x[s * C + t * P : s * C + (t + 1) * P, :]
                    )
                    xt_of[s].append(xti)

            # ---------- one-time constants (scheduled early) ----------
            with tc.high_priority():
                w_bc = sp.tile([P, 3], F32, tag="w_bc")
                nc.sync.dma_start(w_bc[:, :], wt[0:1, :].to_broadcast([P, 3]))

                onesPC = sp.tile([P, C], F32, tag="onesPC")
                nc.vector.memset(onesPC[:, :], 1.0)
                ones128 = sp.tile([P, P], F32, tag="ones128")
                nc.vector.memset(ones128[:, :], 1.0)

                # ident[p, i] = [i == p]
                ident = sp.tile([P, P], F32, tag="ident")
                nc.gpsimd.affine_select(
                    ident[:, :], onesPC[:, 0:P], [[-1, P]], OP.is_equal, 0.0,
                    base=0, channel_multiplier=1,
                )
                # L128[j, m] = [j <= m]  (inclusive lower prefix)
                L128 = sp.tile([P, P], F32, tag="L128")
                nc.gpsimd.affine_select(
                    L128[:, :], onesPC[:, 0:P], [[1, P]], OP.is_ge, 0.0,
                    base=0, channel_multiplier=-1,
                )
                # onehot4_t[k, m] = [k == t]
                onehot4 = sp.tile([SPB, P * NT], F32, tag="onehot4")
                for t in range(NT):
                    nc.gpsimd.affine_select(
                        onehot4[0:NT, t * P : (t + 1) * P],
                        onesPC[0:NT, 0:P],
                        [[0, P]],
                        OP.is_equal,
                        0.0,
                        base=-t,
                        channel_multiplier=1,
                    )

                # Tridiagonal conv weights: z[m] = w0*c[m-1] + w1*c[m] + w2*c[m+1]
                # as a PE matmul out[m,n] = sum_p T[p,m]*comb[p,n]:
                #   T[p,m] = w0*[p==m-1] + w1*[p==m] + w2*[p==m+1]
                scratch = sp.tile([P, P], F32, tag="scratch")
                Tm = sp.tile([P, P], F32, tag="Tm")
                nc.gpsimd.affine_select(
                    scratch[:, :], onesPC[:, 0:P], [[-1, P]], OP.is_equal, 0.0,
                    base=1, channel_multiplier=1,
                )
                nc.vector.tensor_scalar(
                    Tm[:, :], scratch[:, :], w_bc[:, 0:1], None, op0=OP.mult
                )
                nc.vector.scalar_tensor_tensor(
                    out=Tm[:, :], in0=ident[:, :], scalar=w_bc[:, 1:2],
                    op0=OP.mult, in1=Tm[:, :], op1=OP.add,
                )
                nc.gpsimd.affine_select(
                    scratch[:, :], onesPC[:, 0:P], [[-1, P]], OP.is_equal, 0.0,
                    base=-1, channel_multiplier=1,
                )
                nc.vector.scalar_tensor_tensor(
                    out=Tm[:, :], in0=scratch[:, :], scalar=w_bc[:, 2:3],
                    op0=OP.mult, in1=Tm[:, :], op1=OP.add,
                )
                # B0[p,m] = w0*[p==127][m==0]: boundary term from the previous
                # 128-channel block (rhs = comb shifted one column left)
                B0 = sp.tile([P, P], F32, tag="B0")
                nc.gpsimd.affine_select(
                    scratch[:, :], onesPC[:, 0:P], [[-128, P]], OP.is_equal, 0.0,
                    base=-127, channel_multiplier=1,
                )
                nc.vector.tensor_scalar(
                    B0[:, :], scratch[:, :], w_bc[:, 0:1], None, op0=OP.mult
                )
                # B2[p,m] = w2*[p==0][m==127]: boundary from the next block
                B2 = sp.tile([P, P], F32, tag="B2")
                nc.gpsimd.affine_select(
                    scratch[:, :], onesPC[:, 0:P], [[-1, P]], OP.is_equal, 0.0,
                    base=127, channel_multiplier=128,
                )
                nc.vector.tensor_scalar(
                    B2[:, :], scratch[:, :], w_bc[:, 2:3], None, op0=OP.mult
                )

                sum_col = sp.tile([P, SPB * NT], F32, tag="sum_col")
                mx_col = sp.tile([P, SPB * NT], F32, tag="mx_col")
                # zero-guarded padded comb: per sample cols [s*CP .. s*CP+5],
                # data in [s*CP+1 .. s*CP+4], guards stay 0 from the memset
                comb_pad = sp.tile([P, SPB * CP], F32, tag="comb_pad")
                nc.vector.memset(comb_pad[:, :], 0.0)
                score_col = sp.tile([P, SPB * NT], F32, tag="score_col")
                z4s = sp.tile([NT, P], F32, tag="z4s")
                rank_col = sp.tile([P, SPB * NT], F32, tag="rank_col")
                m_col = sp.tile([P, SPB * NT], F32, tag="m_col")
                offf_col = sp.tile([P, SPB * NT], F32, tag="offf_col")
                offi_col = sp.tile([P, SPB * NT], I32, tag="offi_col")

                # stride-0 broadcast outs for accumulator passes: no SBUF
                # write bandwidth or capacity spent on throwaway data
                trash_act = tp.tile([P, 1], F32, tag="trash_act")
                trash_rank = tp.tile([P, 1], F32, tag="trash_rank")

            def phase_a(s):
                """Per-channel sum (ACT) + max (DVE), comb (gpsimd)."""
                cols = slice(s * NT, (s + 1) * NT)
                for t in range(NT):
                    col = slice(s * NT + t, s * NT + t + 1)
                    xti = xt_of[s][t]
                    nc.scalar.activation(
                        trash_act.broadcast_to([P, HW]), xti[:, :], AF.Copy,
                        accum_out=sum_col[:, col],
                    )
                    nc.vector.tensor_reduce(
                        mx_col[:, col],
                        xti[:, :],
                        axis=mybir.AxisListType.X,
                        op=OP.max,
                    )
                nc.vector.scalar_tensor_tensor(
                    out=comb_pad[:, s * CP + 1 : s * CP + 1 + NT],
                    in0=sum_col[:, cols],
                    scalar=1.0 / HW,
                    op0=OP.mult,
                    in1=mx_col[:, cols],
                    op1=OP.add,
                )

            def phase_b(s):
                """z (PE tridiag conv), sigmoid, rank, mask, offsets, scale."""
                cols = slice(s * NT, (s + 1) * NT)
                c0 = s * CP
                z_ps = pp2.tile([P, NT], F32, tag="z_ps")
                nc.tensor.matmul(
                    out=z_ps[:, :], lhsT=Tm[:, :],
                    rhs=comb_pad[:, c0 + 1 : c0 + 1 + NT],
                    start=True, stop=False,
                )
                nc.tensor.matmul(
                    out=z_ps[:, :], lhsT=B0[:, :],
                    rhs=comb_pad[:, c0 : c0 + NT],
                    start=False, stop=False,
                )
                nc.tensor.matmul(
                    out=z_ps[:, :], lhsT=B2[:, :],
                    rhs=comb_pad[:, c0 + 2 : c0 + 2 + NT],
                    start=False, stop=True,
                )
                nc.scalar.activation(score_col[:, cols], z_ps[:, :], AF.Sigmoid)

                # broadcast all 512 scores of this sample to 128 partitions:
                # PE transpose to row form, then block-diagonal matmuls
                z4p = pp.tile([NT, P], F32, tag="z4p")
                nc.tensor.transpose(z4p[:, :], score_col[:, cols], ident[:, :])
                nc.scalar.activation(z4s[:, :], z4p[:, :], AF.Copy)
                zbp = pp.tile([P, C], F32, tag="zbp")
                for t in range(NT):
                    nc.tensor.matmul(
                        out=zbp[:, t * P : (t + 1) * P],
                        lhsT=onehot4[0:NT, t * P : (t + 1) * P],
                        rhs=z4s[:, :],
                        start=True,
                        stop=True,
                    )
                # rank[i] = #{j : score[j] > score[i]}
                for t in range(NT):
                    col = slice(s * NT + t, s * NT + t + 1)
                    nc.vector.tensor_scalar(
                        trash_rank.broadcast_to([P, C]),
                        zbp[:, :],
                        score_col[:, col],
                        None,
                        op0=OP.is_gt,
                        op1=OP.add,
                        accum_out=rank_col[:, col],
                    )
                nc.vector.tensor_scalar(
                    m_col[:, cols], rank_col[:, cols], float(K), None, op0=OP.is_lt
                )
                # inclusive prefix of mask, straight to column form:
                # incl_col[:, t] = sum_{k<t} ones128 @ m_k + L128 @ m_t
                incl_colp = pp2.tile([P, NT], F32, tag="colp")
                nc.tensor.matmul(
                    out=incl_colp[:, 0:NT],
                    lhsT=L128[:, :],
                    rhs=m_col[:, cols],
                    start=True,
                    stop=False,
                )
                for k in range(NT - 1):
                    nc.tensor.matmul(
                        out=incl_colp[:, k + 1 : NT],
                        lhsT=ones128[:, :],
                        rhs=m_col[
                            :, s * NT + k : s * NT + k + 1
                        ].to_broadcast([P, NT - 1 - k]),
                        start=False,
                        stop=(k == NT - 2),
                    )
                # off = incl + BIG + m*(-1 - BIG); unselected stay > bounds
                nc.vector.scalar_tensor_tensor(
                    out=offf_col[:, cols],
                    in0=m_col[:, cols],
                    scalar=float(-1 - BIG),
                    op0=OP.mult,
                    in1=incl_colp[:, :],
                    op1=OP.add,
                )
                nc.vector.tensor_scalar(
                    offi_col[:, cols], offf_col[:, cols], BIG, None, op0=OP.add
                )
                # scale by score and downcast to bf16, engines per SCALE_ENG
                bts = []
                for t in range(NT):
                    col = slice(s * NT + t, s * NT + t + 1)
                    xti = xt_of[s][t]
                    bt = bp.tile([P, HW], BF16, tag="bt")
                    eng = SCALE_ENG[s * NT + t]
                    if eng == "a":
                        nc.scalar.activation(
                            bt[:, :], xti[:, :], AF.Copy,
                            scale=score_col[:, col],
                        )
                    else:
                        nc.gpsimd.tensor_tensor(
                            bt[:, :], xti[:, :],
                            score_col[:, col].broadcast_to([P, HW]),
                            op=OP.mult,
                        )
                    bts.append(bt)
                return bts

            def phase_c(s, bts):
                """Scatter selected (scaled bf16) rows to DRAM."""
                for t in range(NT):
                    col = slice(s * NT + t, s * NT + t + 1)
                    nc.gpsimd.indirect_dma_start(
                        out=outs[s][:, :],
                        out_offset=bass.IndirectOffsetOnAxis(
                            ap=offi_col[:, col], axis=0
                        ),
                        in_=bts[t][:, :],
                        in_offset=None,
                        bounds_check=K - 1,
                        oob_is_err=False,
                    )

            for s in range(SPB):
                phase_a(s)
                bts = phase_b(s)
                phase_c(s, bts)
    if finalize:
        nc.finalize()
    return nc


def kernel(x: np.ndarray, w: np.ndarray) -> np.ndarray:
    assert x.shape == (B, C, H, W) and w.shape == (1, 1, 3)
    if "nc" not in _CACHE:
        _CACHE["nc"] = build_nc()
    nc = _CACHE["nc"]

    xs = np.ascontiguousarray(x, dtype=np.float32).reshape(NCORES, FLAT_IN, HW)
    ws = np.ascontiguousarray(w, dtype=np.float32).reshape(1, 3)
    in_maps = [{"x": xs[i], "w": ws} for i in range(NCORES)]
    res = run_bass_kernel_spmd(nc, in_maps, core_ids=list(range(NCORES)))
    full = []
    for r in res.results:
        full.extend(
            np.asarray(r[f"out{s}"]).astype(np.float32).reshape(1, K, H, W)
            for s in range(SPB)
        )
    return np.concatenate(full, axis=0)


if __name__ == "__main__":
    xin = np.random.randn(B, C, H, W).astype(np.float32)
    win = np.random.randn(1, 1, 3).astype(np.float32)
    o = kernel(xin, win)
    print("kernel out", o.shape, o.dtype, float(np.abs(o).max()))


# revision 15
# speedup vs baseline: 1.0816x; 1.0816x over previous
"""Trainium2 Bass kernel for nn_Attention_56916906606885 (topk channel masking).

Reference computation (per sample b of 32):
  avg[c] = mean(x[b,c,:,:]); mx[c] = max(x[b,c,:,:])          # [512]
  z = conv1d(avg,w,pad=1) + conv1d(mx,w,pad=1)                 # [512] logits
  scores = sigmoid(z)
  top K=256 channels by score, re-sorted by ascending channel index
  out[b,j] = scores[sidx[j]] * x[b, sidx[j]]                   # [256,56,56]

Design (8 NeuronCores, data-parallel over batch, 4 samples/core).
DMA-fabric-bound kernel (16 SDMA engines ~26GB/s each => ~420GB/s/core):
the goal is to keep load DMA (25.7MB f32) and scatter DMA overlapped and
everything else off the critical path.

  - x streamed in [128,3136] channel tiles (recycling pool, 12 bufs).
  - per-channel sum on ScalarE (activation Copy + accum_out), per-channel
    max on VectorE (tensor_reduce) - f32 exact, so the topk selection
    matches the reference bit-for-bit.
  - comb = sum/HW + max written into a zero-guarded padded slab so the
    conv1d boundary terms are plain column-shifted matmul operands - no
    shifted copies needed.
  - z = conv1d(comb) on the PE as tridiagonal matmul + 2 boundary matmuls.
  - selection without sort: rank[i] = #{j: score[j] > score[i]} via DVE
    tensor_scalar against a PE-broadcast score row; mask = rank < K;
    output row = inclusive-prefix-sum of mask (PE matmuls). Unselected
    rows get offsets > bounds so the scatter's OOB-skip drops them.
  - scale pass multiplies by sigmoid score AND downcasts to bf16
    (output tolerance is 2e-2; bf16 rounding is ~1e-3), halving scatter
    bytes. Scales are split across ScalarE/GpSimd to keep VectorE free
    for the 16 max-reduces (its critical load).
  - per-tile indirect DMA scatters selected bf16 rows straight to DRAM;
    host upcasts to f32.
"""

import sys

for _p in ("/opt/trn_rl_repo",):
    if _p not in sys.path:
        sys.path.insert(0, _p)

import numpy as np

import concourse.bass as bass
import concourse.bacc as bacc
import concourse.tile as tile
from concourse import mybir
from concourse.bass_utils import run_bass_kernel_spmd

F32 = mybir.dt.float32
BF16 = mybir.dt.bfloat16
I32 = mybir.dt.int32
AF = mybir.ActivationFunctionType
OP = mybir.AluOpType

B, C, H, W = 32, 512, 56, 56
HW = H * W  # 3136
K = 256
NCORES = 8
SPB = B // NCORES  # 4 samples per core
P = 128
NT = C // P  # 4 channel tiles per sample
FLAT_IN = SPB * C  # 2048 rows per core
BIG = 65536.0  # OOB marker for unselected channels (> any valid row index)
CP = NT + 2  # padded comb columns per sample (zero guard on both sides)

# engine for each of the 16 scale passes, indexed s*NT+t.
# "a"=ScalarE activation (3.0us measured), "g"=GpSimd tensor_tensor with
# stride-0 broadcast score (5.4us measured). VectorE bf16 writes measured
# at 46us (!) so DVE never writes bf16; it keeps the 16 max-reduces.
# Later samples lean ScalarE so the pipeline tail is short.
SCALE_ENG = [
    "a", "g", "a", "g",
    "a", "g", "a", "g",
    "a", "g", "a", "a",
    "a", "a", "a", "g",
]

_CACHE = {}


def build_nc(finalize=True):
    nc = bacc.Bacc()
    x = nc.declare_dram_parameter("x", [FLAT_IN, HW], F32, isOutput=False)
    wt = nc.declare_dram_parameter("w", [1, 3], F32, isOutput=False)
    outs = [
        nc.declare_dram_parameter(f"out{s}", [K, HW], BF16, isOutput=True)
        for s in range(SPB)
    ]

    with tile.TileContext(nc) as tc:
        with (
            tc.tile_pool(name="xp", bufs=12) as xp,
            tc.tile_pool(name="bp", bufs=6) as bp,
            tc.tile_pool(name="small", bufs=1) as sp,
            tc.tile_pool(name="trash", bufs=1) as tp,
            tc.tile_pool(name="psum", bufs=2, space="PSUM") as pp,
            tc.tile_pool(name="psum2", bufs=2, space="PSUM") as pp2,
        ):
            # enqueue every tile load up front: the sync queue is otherwise
            # idle and the DMA engines should never starve. Loads of tiles
            # >= bufs wait (on the sync queue) for the scale pass to free
            # a buffer, which happens long before the data is needed.
            w_bc = sp.tile([P, 3], F32, tag="w_bc")
            nc.sync.dma_start(w_bc[:, :], wt[0:1, :].to_broadcast([P, 3]))
            xt_of = {s: [] for s in range(SPB)}
            for s in range(SPB):
                for t in range(NT):
                    xti = xp.tile([P, HW], F32, tag="xt")
                    nc.sync.dma_start(
                        xti[:, :], x[s * C + t * P : s * C + (t + 1) * P, :]
                    )
                    xt_of[s].append(xti)

            # ---------- one-time constants (scheduled early) ----------
            with tc.high_priority():

                onesPC = sp.tile([P, C], F32, tag="onesPC")
                nc.vector.memset(onesPC[:, :], 1.0)
                ones128 = sp.tile([P, P], F32, tag="ones128")
                nc.vector.memset(ones128[:, :], 1.0)

                # ident[p, i] = [i == p]
                ident = sp.tile([P, P], F32, tag="ident")
                nc.gpsimd.affine_select(
                    ident[:, :], onesPC[:, 0:P], [[-1, P]], OP.is_equal, 0.0,
                    base=0, channel_multiplier=1,
                )
                # Lful[j, m] = [j <= m] - (1+BIG)*[j == m]: inclusive prefix
                # AND the -(1+BIG)*mask offset term, fused in one matrix so
                # the final scatter offset comes out of the prefix matmuls
                # directly: off = Lful@m + blockpref + BIG
                # selected: off = incl-1 in [0,K); unselected: off >= BIG.
                Lful = sp.tile([P, P], F32, tag="Lful")
                nc.gpsimd.affine_select(
                    Lful[:, :], onesPC[:, 0:P], [[1, P]], OP.is_ge, 0.0,
                    base=0, channel_multiplier=-1,
                )
                nc.vector.scalar_tensor_tensor(
                    out=Lful[:, :], in0=ident[:, :], scalar=float(-1 - BIG),
                    op0=OP.mult, in1=Lful[:, :], op1=OP.add,
                )

                # onehot4_t[k, m] = [k == t]
                onehot4 = sp.tile([SPB, P * NT], F32, tag="onehot4")
                for t in range(NT):
                    nc.gpsimd.affine_select(
                        onehot4[0:NT, t * P : (t + 1) * P],
                        onesPC[0:NT, 0:P],
                        [[0, P]],
                        OP.is_equal,
                        0.0,
                        base=-t,
                        channel_multiplier=1,
                    )

                # Tridiagonal conv weights: z[m] = w0*c[m-1] + w1*c[m] + w2*c[m+1]
                # as a PE matmul out[m,n] = sum_p T[p,m]*comb[p,n]:
                #   T[p,m] = w0*[p==m-1] + w1*[p==m] + w2*[p==m+1]
                scratch = sp.tile([P, P], F32, tag="scratch")
                Tm = sp.tile([P, P], F32, tag="Tm")
                nc.gpsimd.affine_select(
                    scratch[:, :], onesPC[:, 0:P], [[-1, P]], OP.is_equal, 0.0,
                    base=1, channel_multiplier=1,
                )
                nc.vector.tensor_scalar(
                    Tm[:, :], scratch[:, :], w_bc[:, 0:1], None, op0=OP.mult
                )
                nc.vector.scalar_tensor_tensor(
                    out=Tm[:, :], in0=ident[:, :], scalar=w_bc[:, 1:2],
                    op0=OP.mult, in1=Tm[:, :], op1=OP.add,
                )
                nc.gpsimd.affine_select(
                    scratch[:, :], onesPC[:, 0:P], [[-1, P]], OP.is_equal, 0.0,
                    base=-1, channel_multiplier=1,
                )
                nc.vector.scalar_tensor_tensor(
                    out=Tm[:, :], in0=scratch[:, :], scalar=w_bc[:, 2:3],
                    op0=OP.mult, in1=Tm[:, :], op1=OP.add,
                )
                # B0[p,m] = w0*[p==127][m==0]: boundary term from the previous
                # 128-channel block (rhs = comb shifted one column left)
                B0 = sp.tile([P, P], F32, tag="B0")
                nc.gpsimd.affine_select(
                    scratch[:, :], onesPC[:, 0:P], [[-128, P]], OP.is_equal, 0.0,
                    base=-127, channel_multiplier=1,
                )
                nc.vector.tensor_scalar(
                    B0[:, :], scratch[:, :], w_bc[:, 0:1], None, op0=OP.mult
                )
                # B2[p,m] = w2*[p==0][m==127]: boundary from the next block
                B2 = sp.tile([P, P], F32, tag="B2")
                nc.gpsimd.affine_select(
                    scratch[:, :], onesPC[:, 0:P], [[-1, P]], OP.is_equal, 0.0,
                    base=127, channel_multiplier=128,
                )
                nc.vector.tensor_scalar(
                    B2[:, :], scratch[:, :], w_bc[:, 2:3], None, op0=OP.mult
                )
                # 1/HW-scaled copies applied to the sum slab: by linearity
                # conv(sum/HW + max) = conv_scaled(sum) + conv(max), which
                # removes the elementwise comb combine from the critical path
                Tms = sp.tile([P, P], F32, tag="Tms")
                B0s = sp.tile([P, P], F32, tag="B0s")
                B2s = sp.tile([P, P], F32, tag="B2s")
                nc.vector.tensor_scalar(
                    Tms[:, :], Tm[:, :], 1.0 / HW, None, op0=OP.mult
                )
                nc.vector.tensor_scalar(
                    B0s[:, :], B0[:, :], 1.0 / HW, None, op0=OP.mult
                )
                nc.vector.tensor_scalar(
                    B2s[:, :], B2[:, :], 1.0 / HW, None, op0=OP.mult
                )

                # zero-guarded padded stat slabs: per sample cols
                # [s*CP .. s*CP+5], data in [s*CP+1 .. s*CP+4], guard cols
                # stay 0 from the memset so the conv boundary reads are 0
                sum_pad = sp.tile([P, SPB * CP], F32, tag="sum_pad")
                nc.vector.memset(sum_pad[:, :], 0.0)
                mx_pad = sp.tile([P, SPB * CP], F32, tag="mx_pad")
                nc.vector.memset(mx_pad[:, :], 0.0)
                score_col = sp.tile([P, SPB * NT], F32, tag="score_col")
                z4s = sp.tile([NT, P], F32, tag="z4s")
                rank_col = sp.tile([P, SPB * NT], F32, tag="rank_col")
                m_col = sp.tile([P, SPB * NT], F32, tag="m_col")
                offi_col = sp.tile([P, SPB * NT], I32, tag="offi_col")

                # stride-0 broadcast outs for accumulator passes: no SBUF
                # write bandwidth or capacity spent on throwaway data
                trash_act = tp.tile([P, 1], F32, tag="trash_act")
                trash_rank = tp.tile([P, 1], F32, tag="trash_rank")

            def phase_a(s):
                """Per-channel sum (ACT) + max (DVE) into the padded slabs."""
                c0 = s * CP
                for t in range(NT):
                    col = slice(c0 + 1 + t, c0 + 2 + t)
                    xti = xt_of[s][t]
                    nc.scalar.activation(
                        trash_act.broadcast_to([P, HW]), xti[:, :], AF.Copy,
                        accum_out=sum_pad[:, col],
                    )
                    nc.vector.tensor_reduce(
                        mx_pad[:, col],
                        xti[:, :],
                        axis=mybir.AxisListType.X,
                        op=OP.max,
                    )

            def phase_b(s):
                """z (PE dual tridiag conv), sigmoid, rank, offsets, scale."""
                cols = slice(s * NT, (s + 1) * NT)
                c0 = s * CP
                z_ps = pp2.tile([P, NT], F32, tag="z_ps")
                nc.tensor.matmul(
                    out=z_ps[:, :], lhsT=Tm[:, :],
                    rhs=mx_pad[:, c0 + 1 : c0 + 1 + NT],
                    start=True, stop=False,
                )
                nc.tensor.matmul(
                    out=z_ps[:, :], lhsT=B0[:, :],
                    rhs=mx_pad[:, c0 : c0 + NT],
                    start=False, stop=False,
                )
                nc.tensor.matmul(
                    out=z_ps[:, :], lhsT=B2[:, :],
                    rhs=mx_pad[:, c0 + 2 : c0 + 2 + NT],
                    start=False, stop=False,
                )
                nc.tensor.matmul(
                    out=z_ps[:, :], lhsT=Tms[:, :],
                    rhs=sum_pad[:, c0 + 1 : c0 + 1 + NT],
                    start=False, stop=False,
                )
                nc.tensor.matmul(
                    out=z_ps[:, :], lhsT=B0s[:, :],
                    rhs=sum_pad[:, c0 : c0 + NT],
                    start=False, stop=False,
                )
                nc.tensor.matmul(
                    out=z_ps[:, :], lhsT=B2s[:, :],
                    rhs=sum_pad[:, c0 + 2 : c0 + 2 + NT],
                    start=False, stop=True,
                )
                nc.scalar.activation(score_col[:, cols], z_ps[:, :], AF.Sigmoid)

                # broadcast all 512 scores of this sample to 128 partitions:
                # PE transpose to row form, then block-diagonal matmuls
                z4p = pp.tile([NT, P], F32, tag="z4p")
                nc.tensor.transpose(z4p[:, :], score_col[:, cols], ident[:, :])
                nc.scalar.activation(z4s[:, :], z4p[:, :], AF.Copy)
                zbp = pp.tile([P, C], F32, tag="zbp")
                for t in range(NT):
                    nc.tensor.matmul(
                        out=zbp[:, t * P : (t + 1) * P],
                        lhsT=onehot4[0:NT, t * P : (t + 1) * P],
                        rhs=z4s[:, :],
                        start=True,
                        stop=True,
                    )
                # rank[i] = #{j : score[j] > score[i]}
                for t in range(NT):
                    col = slice(s * NT + t, s * NT + t + 1)
                    nc.vector.tensor_scalar(
                        trash_rank.broadcast_to([P, C]),
                        zbp[:, :],
                        score_col[:, col],
                        None,
                        op0=OP.is_gt,
                        op1=OP.add,
                        accum_out=rank_col[:, col],
                    )
                nc.vector.tensor_scalar(
                    m_col[:, cols], rank_col[:, cols], float(K), None, op0=OP.is_lt
                )
                # scatter offsets straight out of the prefix matmuls:
                # off[:, t] = Lful @ m_t + sum_{k<t} ones128 @ m_k
                # (selected: exclusive-prefix = output row - BIG;
                #  unselected: prefix + BIG - ... stays >= BIG)
                incl_colp = pp2.tile([P, NT], F32, tag="colp")
                nc.tensor.matmul(
                    out=incl_colp[:, 0:NT],
                    lhsT=Lful[:, :],
                    rhs=m_col[:, cols],
                    start=True,
                    stop=False,
                )
                for k in range(NT - 1):
                    nc.tensor.matmul(
                        out=incl_colp[:, k + 1 : NT],
                        lhsT=ones128[:, :],
                        rhs=m_col[
                            :, s * NT + k : s * NT + k + 1
                        ].to_broadcast([P, NT - 1 - k]),
                        start=False,
                        stop=(k == NT - 2),
                    )
                # int offsets: add back BIG during the PSUM->SBUF i32 cast
                # (PSUM input + immediate: no SBUF read-port pressure)
                nc.vector.tensor_scalar(
                    offi_col[:, cols], incl_colp[:, :], BIG, None, op0=OP.add
                )
                # scale by score and downcast to bf16, engines per SCALE_ENG
                bts = []
                for t in range(NT):
                    col = slice(s * NT + t, s * NT + t + 1)
                    xti = xt_of[s][t]
                    bt = bp.tile([P, HW], BF16, tag="bt")
                    eng = SCALE_ENG[s * NT + t]
                    if eng == "a":
                        nc.scalar.activation(
                            bt[:, :], xti[:, :], AF.Copy,
                            scale=score_col[:, col],
                        )
                    else:
                        nc.gpsimd.tensor_tensor(
                            bt[:, :], xti[:, :],
                            score_col[:, col].broadcast_to([P, HW]),
                            op=OP.mult,
                        )
                    bts.append(bt)
                return bts

            def phase_c(s, bts):
                """Scatter selected (scaled bf16) rows to DRAM."""
                for t in range(NT):
                    col = slice(s * NT + t, s * NT + t + 1)
                    nc.gpsimd.indirect_dma_start(
                        out=outs[s][:, :],
                        out_offset=bass.IndirectOffsetOnAxis(
                            ap=offi_col[:, col], axis=0
                        ),
                        in_=bts[t][:, :],
                        in_offset=None,
                        bounds_check=K - 1,
                        oob_is_err=False,
                    )

            for s in range(SPB):
                phase_a(s)
                bts = phase_b(s)
                phase_c(s, bts)
    if finalize:
        nc.finalize()
    return nc


def kernel(x: np.ndarray, w: np.ndarray) -> np.ndarray:
    assert x.shape == (B, C, H, W) and w.shape == (1, 1, 3)
    if "nc" not in _CACHE:
        _CACHE["nc"] = build_nc()
    nc = _CACHE["nc"]

    xs = np.ascontiguousarray(x, dtype=np.float32).reshape(NCORES, FLAT_IN, HW)
    ws = np.ascontiguousarray(w, dtype=np.float32).reshape(1, 3)
    in_maps = [{"x": xs[i], "w": ws} for i in range(NCORES)]
    res = run_bass_kernel_spmd(nc, in_maps, core_ids=list(range(NCORES)))
    full = []
    for r in res.results:
        full.extend(
            np.asarray(r[f"out{s}"]).astype(np.float32).reshape(1, K, H, W)
            for s in range(SPB)
        )
    return np.concatenate(full, axis=0)


if __name__ == "__main__":
    xin = np.random.randn(B, C, H, W).astype(np.float32)
    win = np.random.randn(1, 1, 3).astype(np.float32)
    o = kernel(xin, win)
    print("kernel out", o.shape, o.dtype, float(np.abs(o).max()))
